# revision 34
# baseline (speedup 1.0000x reference)
"""TRN2 Bass kernel for nn_DerivNet2D.

Reference computation (per sample x in R^2):
    h1 = W1 @ x + b1;  z1 = tanh(h1)            (1024)
    h2 = W2 @ z1 + b2; z2 = tanh(h2)            (512)
    y  = W3 @ z2 + b3                           (1)
    dy/dx_k = W3 @ (dz2 * (W2 @ (dz1 * W1[:,k])))   k = 1, 2
    returns (y, v1, v2) = (y, dy/dx2, -dy/dx1)

Strategy (v6, ~316us vs 377us v5 baseline):
  * Pure data parallel: x split into 8 shards of 8192 samples; weights
    replicated.  SPMD module via run_bass_kernel_spmd.
  * Reverse-mode gradient: B = (-(W2*w3)).T @ (z2^2-1); C = (z1^2-1)*B;
    (y,v) rows = Wyv.T @ [z2-chunks, C-chunks], signs fixed on host.
    w3 is folded into the bwd weights so no per-chunk scale op exists
    between tanh and the bwd GEMM (kills an ACT FIFO head-of-line
    block behind a DVE dependency).
  * All activations/intermediates bf16: ACT writes z1b bf16 directly in
    2-chunk batched tanh ops off a 2-bank h1 psum tile (no f32 z1, no
    SBUF->SBUF cast DMA).  q1 = z1b^2 on GpSimd; q2 = z2^2 and the bwd
    rhs (q2-1) on DVE; C = (q1-1)*B on DVE (bf16 out).
  * yv chain: 4-way PE column tiling at tile_position (0, 32j); group j
    accumulates chunks {z2_j, C_j, C_{j+4}} into pyv rows 32j.. (M
    padded to 32 with zero weight cols so psum is fully initialized).
    One f32r psum->sbuf copy + one selection matmul reduce the groups.
    The tail (C chunks 4-7 + copy + reduce + out DMA) is deferred into
    the NEXT tile's stream where it fills dependency bubbles.
  * bwd chains 0/1 are interleaved across the two B banks so nothing
    needs A(3) until its ACT->DVE latency has passed.  All col-tiled yv
    MMs are clustered (group-3 z2 start rides with the chunk 0-3 batch
    after chain 4): a lone col-tiled MM between full-array MMs breaks
    LDWEIGHTS pull-ahead and costs ~100-200ns per occurrence.
  * L1 with bias folded into the matmul (K=8: bf16 hi/lo of x1,x2,1),
    2-way row tiling (row groups 0/32) into one 2-bank psum tile.
  * Weights prepacked on host into exact SBUF layouts (few large
    contiguous preload DMAs); short pipelined warmup keeps the HAM
    clock-gate at 2.4 GHz through the fill phase.
  * PSUM banks: ph1 2 + ph2 2 + pB 2 + pyv 2 = 8.  Measured steady
    state: 18.2us/tile, bulk MM spacing 216ns (peak), PE ~91% busy.
"""

import numpy as np
from contextlib import ExitStack

import concourse.bacc as bacc
import concourse.mybir as mybir
import concourse.tile as tile
from concourse.bass import ds, ts

F32 = mybir.dt.float32
BF16 = mybir.dt.bfloat16
AF = mybir.ActivationFunctionType
ALU = mybir.AluOpType

NCORES = 8
NX = 65536
NXL = NX // NCORES      # 8192 samples per core
NT = 512                # samples per tile
TILES = NXL // NT       # 16

_CACHE = {}


def build():
    nc = bacc.Bacc(None, target_bir_lowering=False)
    XTb = nc.dram_tensor("XTb", [8, NXL], BF16, kind="ExternalInput")
    W1Tb = nc.dram_tensor("W1Tb", [8, 1024], BF16, kind="ExternalInput")
    W2T = nc.dram_tensor("W2T", [128, 8, 512], BF16, kind="ExternalInput")
    W2N = nc.dram_tensor("W2N", [128, 4, 1024], BF16, kind="ExternalInput")
    WYV = nc.dram_tensor("WYV", [12, 128, 32], BF16, kind="ExternalInput")
    SEL = nc.dram_tensor("SEL", [128, 3], F32, kind="ExternalInput")
    B2S = nc.dram_tensor("B2S", [128, 4], F32, kind="ExternalInput")
    OUT = nc.dram_tensor("out", [3, NXL], F32, kind="ExternalOutput")

    with ExitStack() as ctx:
        tc = ctx.enter_context(tile.TileContext(nc))
        sg = ctx.enter_context(tc.tile_pool(name="sg", bufs=1))
        pxt = ctx.enter_context(tc.tile_pool(name="pxt", bufs=4))
        pz1 = ctx.enter_context(tc.tile_pool(name="pz1", bufs=2))
        pq1 = ctx.enter_context(tc.tile_pool(name="pq1", bufs=2))
        pz2 = ctx.enter_context(tc.tile_pool(name="pz2", bufs=2))
        pq2 = ctx.enter_context(tc.tile_pool(name="pq2", bufs=2))
        pA = ctx.enter_context(tc.tile_pool(name="pA", bufs=2))
        pC = ctx.enter_context(tc.tile_pool(name="pC", bufs=2))
        pyr = ctx.enter_context(tc.tile_pool(name="pyr", bufs=2))
        ph1 = ctx.enter_context(tc.tile_pool(name="ph1", bufs=1, space="PSUM"))
        ph2 = ctx.enter_context(tc.tile_pool(name="ph2", bufs=2, space="PSUM"))
        pB = ctx.enter_context(tc.tile_pool(name="pB", bufs=2, space="PSUM"))
        pyv = ctx.enter_context(tc.tile_pool(name="pyv", bufs=2, space="PSUM"))

        # ---- preload (pure DMA, split across the DGE queues) ----------
        w1t = sg.tile([40, 1024], BF16)
        nc.sync.dma_start(out=w1t[0:8, :], in_=W1Tb[:, :])
        nc.sync.dma_start(out=w1t[32:40, :], in_=W1Tb[:, :])

        wyv = sg.tile([128, 12, 32], BF16)
        nc.gpsimd.dma_start(
            out=wyv, in_=WYV[:, :, :].rearrange("k p m -> p k m")
        )
        selm = sg.tile([128, 3], mybir.dt.float32r)
        nc.gpsimd.dma_start(out=selm, in_=SEL[:, :].bitcast(mybir.dt.float32r))
        b2t = sg.tile([128, 4], F32)
        nc.gpsimd.dma_start(out=b2t, in_=B2S[:, :])

        state = {}

        def emit_xt(T):
            xt = pxt.tile([40, NT], BF16, tag="xt", name="xt")
            nc.sync.dma_start(out=xt[0:8, :], in_=XTb[:, ds(T * NT, NT)])
            nc.sync.dma_start(out=xt[32:40, :], in_=XTb[:, ds(T * NT, NT)])
            state[("xt", T)] = xt

        for _t in range(min(4, TILES)):
            emit_xt(_t)

        # fwd lhsT (prepacked on host): w2t[p, j, c*128+m] = W2[c*128+m, j*128+p]
        w2t = sg.tile([128, 8, 512], BF16)
        nc.sync.dma_start(out=w2t[:, 0:4, :], in_=W2T[:, 0:4, :])
        nc.sync.dma_start(out=w2t[:, 4:8, :], in_=W2T[:, 4:8, :])
        # bwd lhsT (prepacked): w2n[p, c, i*128+m] = -(W2*w3)[c*128+p, i*128+m]
        w2n = sg.tile([128, 4, 1024], BF16)
        nc.scalar.dma_start(out=w2n[:, 0:2, :], in_=W2N[:, 0:2, :])
        nc.scalar.dma_start(out=w2n[:, 2:4, :], in_=W2N[:, 2:4, :])

        # ---- PE warmup: pipelined dummy MMs so the HAM clock-gate
        # reaches 2.4 GHz while the weight DMAs stream in --------------
        warm = sg.tile([128, NT], BF16)
        nc.vector.memset(warm, 0.0)
        for _ in range(16):
            pw = ph2.tile([128, NT], F32, tag="h2", name="p2")
            nc.tensor.matmul(pw, warm[:, 0:128], warm, start=True, stop=True)

        def emit_l1_pair(T, c0):
            """Two L1 chunk MMs (row groups 0/32) into one 2-bank psum
            tile; one batched tanh -> z1b bf16."""
            xt = state[("xt", T)]
            if c0 == 0:
                state[("z1b", T)] = pz1.tile(
                    [128, 8, NT], BF16, tag="z1b", name="z1b"
                )
            z1b = state[("z1b", T)]
            p1 = ph1.tile([128, 2, NT], F32, tag="h1", name="p1")
            for s, g in ((0, 0), (1, 32)):
                nc.tensor.matmul(
                    p1[:, s, :],
                    w1t[g : g + 8, ts(c0 + s, 128)],
                    xt[g : g + 8, :],
                    start=True, stop=True,
                    tile_position=(g, 0),
                )
            nc.scalar.activation(z1b[:, ds(c0, 2), :], p1, AF.Tanh)
            if c0 == 6:
                state[T] = state.pop(("z1b", T))

        def emit_yv_tail(Tm):
            """Finish tile Tm's yv: C chunks 4-7 into the col groups,
            psum->sbuf copy, selection reduce, output DMA."""
            st = state.pop(("yvtail", Tm), None)
            if st is None:
                return
            pyvt, C = st
            for k in (0, 1, 2, 3):
                nc.tensor.matmul(
                    pyvt[ds(32 * k, 32), :], wyv[:, 4 + k, :],
                    C[:, 4 + k, :],
                    start=False, stop=True, skip_group_check=True,
                    tile_position=(0, 32 * k),
                )
            yvr = pyr.tile([128, NT], mybir.dt.float32r, tag="yvr", name="yvr")
            nc.vector.tensor_copy(yvr, pyvt)
            state[("yvred", Tm)] = yvr

        def emit_yv_reduce(Tm):
            yvr = state.pop(("yvred", Tm), None)
            if yvr is None:
                return
            pyo = pyv.tile([3, NT], F32, tag="yv", name="pyo")
            nc.tensor.matmul(pyo, selm, yvr, start=True, stop=True)
            yvs = pyr.tile([3, NT], F32, tag="yvs", name="yvs")
            nc.vector.tensor_copy(yvs, pyo)
            nc.gpsimd.dma_start(
                out=OUT[:, ds(Tm * NT, NT)], in_=yvs[0:3, :]
            )

        for T in range(TILES + 1):
            if 4 <= T + 3 < TILES:
                emit_xt(T + 3)
            if T == 0:
                # L1 pairs with warmup MMs between them: PE streams while
                # each pair's tanh drains and w2t finishes loading
                for c0 in (0, 2, 4, 6):
                    emit_l1_pair(0, c0)
                    for _ in range(6):
                        pw = ph2.tile([128, NT], F32, tag="h2", name="p2")
                        nc.tensor.matmul(pw, warm[:, 0:128], warm,
                                         start=True, stop=True)
                continue

            # ---------------- rest of tile T-1 ------------------------
            Tm = T - 1
            z1b = state.pop(Tm)
            state.pop(("xt", Tm), None)

            # previous-previous tile's yv tail: ready PE work that fills
            # the front of this tile while its own deps resolve
            emit_yv_tail(T - 2)

            # q1 = z1b^2 on GpSimd, two halves, bf16
            q1 = pq1.tile([128, 8, NT], BF16, tag="q1", name="q1")
            for h in range(2):
                nc.gpsimd.tensor_mul(
                    q1[:, ds(4 * h, 4), :],
                    z1b[:, ds(4 * h, 4), :],
                    z1b[:, ds(4 * h, 4), :],
                )

            # fwd: h2 = W2 @ z1 + b2; per chunk: tanh(+bias), q2 = z2^2,
            # A = q2*(-w3) + w3
            z2 = pz2.tile([128, 4, NT], BF16, tag="z2", name="z2")
            q2 = pq2.tile([128, 4, NT], BF16, tag="q2", name="q2")
            A = pA.tile([128, 4, NT], BF16, tag="A", name="A")
            for c in range(4):
                p2 = ph2.tile([128, NT], F32, tag="h2", name="p2")
                for j in range(8):
                    nc.tensor.matmul(
                        p2,
                        w2t[:, j, ds(c * 128, 128)],
                        z1b[:, j, :],
                        start=(j == 0), stop=(j == 7),
                    )
                nc.scalar.activation(
                    z2[:, c, :], p2, AF.Tanh, bias=b2t[:, c : c + 1]
                )
                nc.vector.tensor_mul(q2[:, c, :], z2[:, c, :], z2[:, c, :])
                # A = q2 - 1 = -dz2; w3 and the sign are folded into W2N
                nc.vector.tensor_scalar_sub(A[:, c, :], q2[:, c, :], 1.0)

            # yv accum psum: 4 col-groups at partitions 32j..; z2 part.
            # Groups 0-2 first; group 3 (needs z2 chunk 3) after filler.
            pyvt = pyv.tile([128, NT], F32, tag="yv", name="pyvt")
            for k in range(3):
                nc.tensor.matmul(
                    pyvt[ds(32 * k, 32), :], wyv[:, 8 + k, :], z2[:, k, :],
                    start=True, stop=False, skip_group_check=True,
                    tile_position=(0, 32 * k),
                )
            emit_yv_reduce(T - 2)

            # bwd chains; chains 0 and 1 interleaved across the two B
            # banks so no MM needs A(3) until its latency is covered
            C = pC.tile([128, 8, NT], BF16, tag="C", name="C")
            pb0 = pB.tile([128, NT], F32, tag="B", name="pb")
            pb1 = pB.tile([128, NT], F32, tag="B", name="pb")
            for c in range(3):
                nc.tensor.matmul(pb0, w2n[:, c, ds(0, 128)], A[:, c, :],
                                 start=(c == 0), stop=False)
            for c in range(3):
                nc.tensor.matmul(pb1, w2n[:, c, ds(128, 128)], A[:, c, :],
                                 start=(c == 0), stop=False)
            # L1 pair 0 here: ready full-rate work that covers the last
            # ~200ns of A(3)'s tanh->square->sub latency before the stops
            if T < TILES:
                emit_l1_pair(T, 0)
            nc.tensor.matmul(pb0, w2n[:, 3, ds(0, 128)], A[:, 3, :],
                             start=False, stop=True)
            nc.vector.scalar_tensor_tensor(
                out=C[:, 0, :], in0=q1[:, 0, :], scalar=1.0, in1=pb0,
                op0=ALU.subtract, op1=ALU.mult,
            )
            nc.tensor.matmul(pb1, w2n[:, 3, ds(128, 128)], A[:, 3, :],
                             start=False, stop=True)
            nc.vector.scalar_tensor_tensor(
                out=C[:, 1, :], in0=q1[:, 1, :], scalar=1.0, in1=pb1,
                op0=ALU.subtract, op1=ALU.mult,
            )
            for i in range(2, 8):
                pb = pB.tile([128, NT], F32, tag="B", name="pb")
                for c in range(4):
                    nc.tensor.matmul(
                        pb,
                        w2n[:, c, ds(i * 128, 128)],
                        A[:, c, :],
                        start=(c == 0), stop=(c == 3),
                    )
                nc.vector.scalar_tensor_tensor(
                    out=C[:, i, :], in0=q1[:, i, :], scalar=1.0, in1=pb,
                    op0=ALU.subtract, op1=ALU.mult,
                )
                if i % 2 == 0 and T < TILES:
                    emit_l1_pair(T, i)
                if i == 4:
                    # group-3 z2 part (start) + yv C chunks 0-3, clustered
                    nc.tensor.matmul(
                        pyvt[ds(96, 32), :], wyv[:, 11, :], z2[:, 3, :],
                        start=True, stop=False, skip_group_check=True,
                        tile_position=(0, 96),
                    )
                    for k in range(4):
                        nc.tensor.matmul(
                            pyvt[ds(32 * k, 32), :], wyv[:, k, :],
                            C[:, k, :],
                            start=False, stop=False, skip_group_check=True,
                            tile_position=(0, 32 * k),
                        )
            state[("yvtail", Tm)] = (pyvt, C)

        # drain the last two tiles' yv tails
        emit_yv_tail(TILES - 2)
        emit_yv_reduce(TILES - 2)
        emit_yv_tail(TILES - 1)
        emit_yv_reduce(TILES - 1)

    nc.compile()
    return nc


def prep_inputs(x_shard, W1, b1, W2, b2, W3, b3):
    """Host-side layout prep for one core's shard."""
    import ml_dtypes

    f32 = np.float32
    bf16 = ml_dtypes.bfloat16
    # L1 in bf16 with hi/lo splitting: K=8 rows
    #   lhsT: [w1a_hi, w1a_hi, w1a_lo, w1b_hi, w1b_hi, w1b_lo, b1_hi, b1_lo]
    #   rhs:  [x1hi,   x1lo,   x1hi,   x2hi,   x2lo,   x2hi,   1,     1   ]
    x1 = x_shard[:, 0].astype(f32)
    x2 = x_shard[:, 1].astype(f32)
    x1hi = x1.astype(bf16)
    x1lo = (x1 - x1hi.astype(f32)).astype(bf16)
    x2hi = x2.astype(bf16)
    x2lo = (x2 - x2hi.astype(f32)).astype(bf16)
    one = np.ones(NXL, bf16)
    xtb = np.stack([x1hi, x1lo, x1hi, x2hi, x2lo, x2hi, one, one])
    w1a = W1[:, 0].astype(f32)
    w1b = W1[:, 1].astype(f32)
    w1a_hi = w1a.astype(bf16)
    w1a_lo = (w1a - w1a_hi.astype(f32)).astype(bf16)
    w1b_hi = w1b.astype(bf16)
    w1b_lo = (w1b - w1b_hi.astype(f32)).astype(bf16)
    b1hi = b1.astype(bf16)
    b1lo = (b1.astype(f32) - b1hi.astype(f32)).astype(bf16)
    w1tb = np.stack([w1a_hi, w1a_hi, w1a_lo, w1b_hi, w1b_hi, w1b_lo, b1hi, b1lo])
    # yv weights: C-part (chunks 0..7) from W1 columns (signs fixed),
    # z2-part (chunks 8..11) from W3.
    wyv = np.zeros((12, 128, 32), f32)
    for i in range(8):
        blk = W1[i * 128 : (i + 1) * 128]
        wyv[i, :, 0] = blk[:, 1]
        wyv[i, :, 1] = blk[:, 0]
    for c in range(4):
        wyv[8 + c, :, 2] = W3[0, c * 128 : (c + 1) * 128]
    sel = np.zeros((128, 3), f32)
    for j in range(4):
        for m in range(3):
            sel[32 * j + m, m] = 1.0
    b2s = np.ascontiguousarray(b2.reshape(4, 128).T)
    return {
        "XTb": np.ascontiguousarray(xtb),
        "W1Tb": np.ascontiguousarray(w1tb),
        "W2T": np.ascontiguousarray(
            W2.reshape(4, 128, 8, 128).transpose(3, 2, 0, 1).reshape(128, 8, 512)
        ).astype(bf16),
        "W2N": np.ascontiguousarray(
            (-(W2 * W3[0][:, None])).reshape(4, 128, 1024).transpose(1, 0, 2)
        ).astype(bf16),
        "WYV": wyv.astype(bf16),
        "SEL": sel,
        "B2S": np.ascontiguousarray(b2s.astype(f32)),
    }


def postprocess(o, b3):
    """o: [3, NXL] -> (y, v1, v2) for the shard."""
    v1 = -o[0]
    v2 = o[1]
    y = o[2] + b3[0]
    return y, v1, v2


def kernel(x, W1, b1, W2, b2, W3, b3):
    from concourse.bass_utils import run_bass_kernel_spmd

    if "nc" not in _CACHE:
        _CACHE["nc"] = build()
    nc = _CACHE["nc"]

    x = np.asarray(x, dtype=np.float32)
    W1 = np.asarray(W1, dtype=np.float32)
    b1 = np.asarray(b1, dtype=np.float32)
    W2 = np.asarray(W2, dtype=np.float32)
    b2 = np.asarray(b2, dtype=np.float32)
    W3 = np.asarray(W3, dtype=np.float32)
    b3 = np.asarray(b3, dtype=np.float32)

    shards = np.split(x, NCORES, axis=0)
    in_maps = [
        prep_inputs(shards[c], W1, b1, W2, b2, W3, b3) for c in range(NCORES)
    ]
    _CACHE["in_maps"] = in_maps

    res = run_bass_kernel_spmd(nc, in_maps, core_ids=list(range(NCORES)))
    ys, v1s, v2s = [], [], []
    for c in range(NCORES):
        y, v1, v2 = postprocess(res.results[c]["out"], b3)
        ys.append(y)
        v1s.append(v1)
        v2s.append(v2)
    y = np.concatenate(ys).reshape(NX, 1).astype(np.float32)
    v1 = np.concatenate(v1s).reshape(NX, 1).astype(np.float32)
    v2 = np.concatenate(v2s).reshape(NX, 1).astype(np.float32)
    return (y, v1, v2)
